# revision 1
# baseline (speedup 1.0000x reference)
"""Trainium2 Bass kernel for nn_LConvBilin (lattice gauge bilinear conv).

Full on-device compute, 8-core SPMD: V=16384 sites split contiguously across
8 NeuronCores (2048 sites/core, 16 tiles of 128 sites on SBUF partitions).

Per-tile pipeline:
  stage A (DVE): gauge transports T_am = u_a w~_am u_a^dag as broadcast-AP
      tensor_tensor products written sign-folded into big tiles, then single
      contiguous-innermost reduces over (term, contraction-dim);
      results land in TALL [128,360] = t_w channel-major (ch, r, c, q).
  gather+transpose (ACT+PE): TALL {r,q}-pair blocks -> channel-on-partition
      tiles TQ [80,128] / TQD [41,128] (41st row = ones, for the unit term).
  stage C (PE): M-contraction with the per-site DATA as the stationary
      operand and the weight matrix [80/41, 72] as the moving operand ->
      M[site, (u,v,c)] lands sites-on-partitions in PSUM; ACT evacuates into
      MS [128,648] laid out (t, u, v, j, c) for stage E.
  stage E (DVE): bilinear out[u] = M[u,8] + sum_v w_v M[u,v] + w_v^dag
      M[u,v+4] as 8 batched products against host-replicated w-tiles (DMA'd),
      then one XY-reduce over (term, v, j) per complex part.

Host does layout only: reshapes/rolls inputs into per-core arrays (shifted
w-fields, replicated/negated w-tiles, weight matrices with t_w/t_w^dag
c-sign folding and the eye row) and unpermutes the output.

kernel(x, weight) takes FULL inputs and returns the FULL output.
Known-good: rel err ~3e-7 vs reference; HW exec ~373us (profiled via
run_bass_kernel_spmd(trace=True)).
"""
import re
import sys

import numpy as np

sys.path.insert(0, "/opt/trn_rl_repo")

DIMS = (16, 16, 8, 8)
V = 16384
N_CORES = 8
S = V // N_CORES            # 2048 sites per core
NT = S // 128               # 16 site-tiles per core
PAIRS = [(0, 1), (0, 2), (1, 2)]

_CACHE = {}
SPLIT_WAITS = True


# ---------------------------------------------------------------- tile fixes
def _apply_tile_fixes():
    """This walrus build allows very few semaphore waits per instruction.
    (a) Replace TileContext._drain_and_barrier with a version that splits the
        global-clock wait across single-wait sync NOPs.
    (b) Post-pass splitting any instruction's waits beyond 1 onto same-engine
        NOPs inserted before it."""
    if _CACHE.get("fixed"):
        return
    from concourse.tile import TileContext
    from concourse.vector_clock import ScopedClock, VectorClock

    def _clock_values(vc):
        m = re.match(r"VectorClock\(\[(.*)\]\)", repr(vc))
        return [int(x) for x in m.group(1).split(",")]

    def _drain_and_barrier_split(self, tick_clock, wait_clock):
        vals = _clock_values(tick_clock.global_clock)
        for p, val in [(p, v) for p, v in enumerate(vals) if v > 0]:
            v = VectorClock()
            v.require_at_least(p, val)
            nop_inst = self.nc.sync.nop(nofuse=True, hint="drain_split_wait")
            wait_clock.add_sem_waits(nop_inst.ins, ScopedClock({None: v}))
        self.nc.sync.drain()
        self.nc.all_engine_barrier()
        assert self.sems is not None
        popped = self.nc._tile_sem_poison_stack.pop()
        assert popped is self._sem_poison
        self.nc.clear_and_free_semaphores(list(self.sems.allocated().values()))
        self.nc.all_engine_barrier()

    TileContext._drain_and_barrier = _drain_and_barrier_split
    _CACHE["fixed"] = True


def _split_sync_waits(nc, cap=1):
    import concourse.mybir as mybir

    for fn in nc.m.functions:
        for bb in fn.blocks:
            out = []
            for inst in bb.instructions:
                si = inst.sync_info
                if si is not None and si.on_wait and len(si.on_wait) > cap:
                    waits = list(si.on_wait)
                    for i in range(cap, len(waits), cap):
                        nop = mybir.InstNoOp(
                            name=f"{inst.name}-wsplit{i}", ins=[], outs=[]
                        )
                        nop.engine = inst.engine
                        nop.sync_info = mybir.SyncInfo(
                            on_wait=waits[i : i + cap], on_update=[]
                        )
                        nop.bass_nofuse = True
                        out.append(nop)
                    si.on_wait = waits[:cap]
                out.append(inst)
            bb.instructions = out


# ---------------------------------------------------------------- program
def _build_program():
    import concourse.bass as bass
    import concourse.mybir as mybir
    from concourse.masks import make_identity
    from concourse.tile import TileContext

    _apply_tile_fixes()
    F32 = mybir.dt.float32
    MULT = mybir.AluOpType.mult
    ADD = mybir.AluOpType.add
    SUB = mybir.AluOpType.subtract

    nc = bass.Bass()
    XU = nc.dram_tensor("XU", [S, 72], F32, kind="ExternalInput")
    WSA = nc.dram_tensor("WSA", [S, 288], F32, kind="ExternalInput")
    XWS = nc.dram_tensor("XWS", [S, 72], F32, kind="ExternalInput")
    WTSD = nc.dram_tensor("WTSD", [81, 648], F32, kind="ExternalInput")
    WBH = nc.dram_tensor("WBH", [S, 1728], F32, kind="ExternalInput")
    WBNH = nc.dram_tensor("WBNH", [S, 864], F32, kind="ExternalInput")
    YS = nc.dram_tensor("YS", [S, 72], F32, kind="ExternalOutput")

    def AP(t, off, dims):
        return bass.AP(t.tensor, t.offset + off, [list(t.ap[0])] + dims)

    with TileContext(nc) as tc:
        with (
            tc.tile_pool(name="const", bufs=1) as cpool,
            tc.tile_pool(name="work", bufs=3) as pool,
            tc.tile_pool(name="big", bufs=3) as bigpool,
            tc.tile_pool(name="ps_tr", bufs=2, space="PSUM") as ps_tr,
            tc.tile_pool(name="ps_mm", bufs=4, space="PSUM") as ps_mm,
        ):
            ident = cpool.tile([128, 128], F32)
            make_identity(nc, ident[:, :])
            wtsb = cpool.tile([81, 648], F32)
            nc.sync.dma_start(wtsb[:, :], WTSD[:, :])

            def emit_front(t):
                rows = slice(t * 128, (t + 1) * 128)
                ut = pool.tile([128, 72], F32, tag="ut")
                wt = pool.tile([128, 288], F32, tag="wt")
                tall = pool.tile([128, 360], F32, tag="tall")
                nc.sync.dma_start(ut[:, :], XU[rows, :])
                nc.sync.dma_start(wt[:, :], WSA[rows, :])
                nc.sync.dma_start(tall[:, 0:72], XWS[rows, :])

                # ---- stage A: transports ----
                # WREP [128,864] (a,i,(j,c,m,k)) <- wt (a,(r,c,m,q)) bcast i
                wrep = bigpool.tile([128, 864], F32, tag="wrep")
                for a in range(4):
                    nc.sync.dma_start(
                        AP(wrep, 216 * a, [[1, 216]]),
                        AP(wt, 72 * a, [[0, 3], [1, 72]]),
                    )
                # UTN [128,36] = -u_imag (a,r,q)  (ACT)
                utn = pool.tile([128, 36], F32, tag="utn")
                nc.scalar.mul(
                    AP(utn, 0, [[1, 36]]), AP(ut, 36, [[1, 36]]), -1.0
                )
                # P1 products into big tiles [128,864] (a,i,term,j,m,k);
                # signs folded (term1 of IM uses -uI); merged (term,j)-reduce
                p1re = bigpool.tile([128, 864], F32, tag="p1re")
                p1im = bigpool.tile([128, 864], F32, tag="p1im")
                for big, term, cu, cw in (
                    (p1re, 0, 0, 0),
                    (p1re, 1, None, 1),
                    (p1im, 0, 0, 1),
                    (p1im, 1, 1, 0),
                ):
                    if cu is None:
                        in0 = AP(utn, 0, [[3, 12], [0, 12], [1, 3]])
                    else:
                        in0 = AP(ut, 36 * cu, [[3, 12], [0, 12], [1, 3]])
                    nc.vector.tensor_tensor(
                        out=AP(big, 3 * term, [[72, 12], [6, 12], [1, 3]]),
                        in0=in0,
                        in1=AP(wrep, 36 * cw, [[72, 12], [3, 12], [1, 3]]),
                        op=MULT,
                    )
                # reduce over (term,j) -> VT-all [128,288] (a,i,m,c,k)
                vta = pool.tile([128, 288], F32, tag="vta")
                for src_, c in ((p1re, 0), (p1im, 1)):
                    nc.vector.tensor_reduce(
                        out=AP(vta, 3 * c, [[24, 12], [6, 4], [1, 3]]),
                        in_=AP(src_, 0, [[6, 144], [1, 6]]),
                        axis=mybir.AxisListType.X,
                        op=ADD,
                    )
                # VT2 [128,288] (a,m,i,c,k): 4 per-axis transmute copies
                vt2 = pool.tile([128, 288], F32, tag="vt2")
                for a in range(4):
                    nc.vector.tensor_copy(
                        AP(vt2, 72 * a, [[18, 4], [6, 3], [1, 6]]),
                        AP(vta, 72 * a, [[6, 4], [24, 3], [1, 6]]),
                    )
                # P2: T = Vt u^dag: Re = VtR uR + VtI uI ; Im = VtI uR + VtR(-uI)
                p2re = bigpool.tile([128, 864], F32, tag="p2re")
                p2im = bigpool.tile([128, 864], F32, tag="p2im")
                for big, term, cv, cu in (
                    (p2re, 0, 0, 0),
                    (p2re, 1, 1, 1),
                    (p2im, 0, 1, 0),
                    (p2im, 1, 0, None),
                ):
                    if cu is None:
                        in1 = AP(utn, 0, [[9, 4], [0, 12], [1, 9]])
                    else:
                        in1 = AP(ut, 36 * cu, [[9, 4], [0, 12], [1, 9]])
                    nc.vector.tensor_tensor(
                        out=AP(big, 3 * term, [[6, 144], [1, 3]]),
                        in0=AP(vt2, 3 * cv, [[6, 48], [0, 3], [1, 3]]),
                        in1=in1,
                        op=MULT,
                    )
                # reduce over (term,k) -> TALL T-part (ch=4+4a+m, r=i, c, q=l)
                for src_, c in ((p2re, 0), (p2im, 1)):
                    nc.vector.tensor_reduce(
                        out=AP(tall, 72 + 3 * c, [[18, 16], [6, 3], [1, 3]]),
                        in_=AP(src_, 0, [[6, 144], [1, 6]]),
                        axis=mybir.AxisListType.X,
                        op=ADD,
                    )

                # ---- stage-E w-replications: host-precomputed, DMA in ----
                wbt = bigpool.tile([128, 1728], F32, tag="wbt")
                wbnt = bigpool.tile([128, 864], F32, tag="wbnt")
                nc.sync.dma_start(wbt[:, :], WBH[rows, :])
                nc.sync.dma_start(wbnt[:, :], WBNH[rows, :])

                # ---- transposes: TALL -> channel-on-partition TQ tiles ----
                # (transpose streaming operand needs a single free dim: gather
                # the pair-block into a contiguous tile first, on ACT)
                tq = []
                for pi, (p_, P_) in enumerate(PAIRS):
                    gat = pool.tile([128, 80], F32, tag="gat")
                    nc.scalar.copy(
                        gat[:, :],
                        AP(tall, 6 * p_ + P_, [[3, 2], [5 * (P_ - p_), 2], [18, 20]]),
                    )
                    pt = ps_tr.tile([80, 128], F32, tag="pt")
                    nc.tensor.transpose(pt[:, :], gat[:, :], ident[:, :])
                    sq = pool.tile([80, 128], F32, tag=f"tq{pi}")
                    nc.scalar.copy(sq[:, :], pt[:, :])
                    tq.append(sq)
                tqd = []
                for r in range(3):
                    gat = pool.tile([128, 41], F32, tag="gatd")
                    nc.scalar.copy(
                        gat[:, 0:40], AP(tall, 7 * r, [[3, 2], [18, 20]])
                    )
                    nc.gpsimd.memset(gat[:, 40:41], 1.0)
                    pt = ps_tr.tile([80, 128], F32, tag="pt")
                    nc.tensor.transpose(pt[0:41, :], gat[:, :], ident[:, :])
                    sq = pool.tile([41, 128], F32, tag=f"tqd{r}")
                    nc.scalar.copy(sq[:, :], pt[0:41, :])
                    tqd.append(sq)

                # ---- stage C: M-contraction (data stationary, weights move) ----
                ms = pool.tile([128, 648], F32, tag="ms")
                pair_idx = {(0, 1): 0, (0, 2): 1, (1, 2): 2}
                for r in range(3):
                    for q in range(3):
                        rq = r * 3 + q
                        mm = ps_mm.tile([128, 72], F32, tag="mm")
                        if r == q:
                            lhs = tqd[r][0:41, :]
                            rhs = wtsb[0:41, rq * 72 : (rq + 1) * 72]
                        else:
                            pi = pair_idx[(min(r, q), max(r, q))]
                            lhs = tq[pi][0:80, :]
                            rhs = wtsb[0:80, rq * 72 : (rq + 1) * 72]
                        nc.tensor.matmul(mm[:, :], lhs, rhs, start=True, stop=True)
                        # mm cols (u,v,c) -> MS2 (t=q, u, v, j=r, c)
                        nc.scalar.copy(
                            AP(ms, q * 108 + r, [[27, 4], [3, 9], [324, 2]]),
                            mm[:, :],
                        )

                return rows, ms, wbt, wbnt

            def emit_back(state):
                rows, ms, wbt, wbnt = state
                # ---- stage E: bilinear ----
                # 8 batched products into RE/IM-big [128, 1728] (term, v,j,t,u,i)
                rebig = bigpool.tile([128, 1728], F32, tag="rebig")
                imbig = bigpool.tile([128, 1728], F32, tag="imbig")
                M_IN1 = lambda vb, cm: AP(ms, 324 * cm + 3 * vb, [[27, 12], [0, 3], [1, 12]])
                terms = [
                    (rebig, 0, wbt, 0),       # norm rr:  wb c0
                    (rebig, 1, wbnt, 0),      # norm -ii: -wb c1
                    (rebig, 2, wbt, 864),     # dag rr:   wbd c0
                    (rebig, 3, wbt, 864 + 432),  # dag +ii: wbd c1
                    (imbig, 0, wbt, 0),       # norm ri:  wb c0
                    (imbig, 1, wbt, 432),     # norm ir:  wb c1
                    (imbig, 2, wbt, 864),     # dag ri:   wbd c0
                    (imbig, 3, wbnt, 432),    # dag -ir:  -wbd c1
                ]
                cms = [0, 1, 0, 1, 1, 0, 1, 0]
                vbs = [0, 0, 4, 4, 0, 0, 4, 4]
                for k_, (big, term, w_, woff) in enumerate(terms):
                    nc.vector.tensor_tensor(
                        out=AP(big, 432 * term, [[12, 36], [1, 12]]),
                        in0=AP(w_, woff, [[36, 12], [12, 3], [1, 12]]),
                        in1=M_IN1(vbs[k_], cms[k_]),
                        op=MULT,
                    )
                # reduce over (term, v, j) -> OUT (t,u,i,c)
                outt = pool.tile([128, 72], F32, tag="outt")
                for src_, c in ((rebig, 0), (imbig, 1)):
                    nc.vector.tensor_reduce(
                        out=AP(outt, c, [[24, 3], [6, 4], [2, 3]]),
                        in_=AP(src_, 0, [[12, 36], [432, 4], [1, 12]]),
                        axis=mybir.AxisListType.XY,
                        op=ADD,
                    )
                    # += M[u,8]
                    nc.vector.tensor_tensor(
                        out=AP(outt, c, [[24, 3], [6, 4], [2, 3]]),
                        in0=AP(outt, c, [[24, 3], [6, 4], [2, 3]]),
                        in1=AP(ms, 324 * c + 24, [[108, 3], [27, 4], [1, 3]]),
                        op=ADD,
                    )
                nc.sync.dma_start(YS[rows, :], outt[:, :])

            prev = None
            for t in range(NT):
                st = emit_front(t)
                if prev is not None:
                    emit_back(prev)
                prev = st
            emit_back(prev)
    if SPLIT_WAITS:
        _split_sync_waits(nc)
    return nc


# ---------------------------------------------------------------- host prep
def _host_prep(x, weight):
    x = np.ascontiguousarray(x, dtype=np.float32)
    weight = np.ascontiguousarray(weight, dtype=np.float32)
    xu = x[0, :, :4].transpose(0, 4, 1, 2, 3).reshape(V, 72)        # (c,a,r,q)
    wgrid = x[0, :, 4:].reshape(DIMS + (4, 3, 3, 2))
    ws = []
    for a in range(4):
        r = np.roll(wgrid, -1, axis=a).reshape(V, 4, 3, 3, 2)
        ws.append(r.transpose(0, 4, 1, 3, 2).reshape(V, 72))        # (c,m,q,r)
    wsa = np.concatenate([w[:, None, :] for w in ws], axis=1).reshape(V, 288)
    xws = x[0, :, 4:].transpose(0, 1, 2, 4, 3).reshape(V, 72)       # (m,r,c,q)
    # replicated w-tiles for stage E: wb[c,v,j,tu,i]=w[v][i,j,c],
    # wbd[c,v,j,tu,i]=w[v][j,i,c]; negated c1-halves for sign folding
    w_ = x[0, :, 4:]
    arr1 = w_.transpose(0, 4, 1, 3, 2)   # [V,c,v,j,i] = w[v][i,j,c]
    arr2 = w_.transpose(0, 4, 1, 2, 3)   # [V,c,v,j,i] = w[v][j,i,c]
    wb = np.broadcast_to(
        arr1.transpose(0, 1, 4, 2, 3)[:, :, None], (V, 2, 12, 3, 4, 3)
    )
    wbd = np.broadcast_to(
        arr2.transpose(0, 1, 4, 2, 3)[:, :, None], (V, 2, 12, 3, 4, 3)
    )
    wbh = np.concatenate(
        [wb.reshape(V, 864), wbd.reshape(V, 864)], axis=1
    ).astype(np.float32)
    wbnh = np.concatenate(
        [-wb[:, 1].reshape(V, 432), -wbd[:, 1].reshape(V, 432)], axis=1
    ).astype(np.float32)

    o1 = weight[:, :, :20]      # [u,v,ch]
    o2 = weight[:, :, 20:40]
    o3 = weight[:, :, 40]
    # WTSD [80, 648]: 9 slices rq=(r*3+q), cols (u,v,cout), rows:
    #  off-diag pair {p<P}: row = c*40 + ord*20 + ch, ord0=(p,P), ord1=(P,p)
    #  diag r: row = c*20 + ch (rows 40:80 zero)
    wtsd = np.zeros((81, 648), np.float32)
    for r in range(3):
        for q in range(3):
            rq = r * 3 + q
            blk = np.zeros((81, 4, 9, 2), np.float32)
            if r == q:
                for c in range(2):
                    sgn = 1.0 if c == 0 else -1.0
                    for ch in range(20):
                        blk[c * 20 + ch, :, :, c] = o1[:, :, ch] + sgn * o2[:, :, ch]
                blk[40, :, :, 0] = o3
            else:
                p_, P_ = min(r, q), max(r, q)
                my_ord = 0 if (r, q) == (p_, P_) else 1
                for c in range(2):
                    sgn = 1.0 if c == 0 else -1.0
                    for ch in range(20):
                        blk[c * 40 + my_ord * 20 + ch, :, :, c] = o1[:, :, ch]
                        blk[c * 40 + (1 - my_ord) * 20 + ch, :, :, c] = sgn * o2[:, :, ch]
            wtsd[:, rq * 72 : (rq + 1) * 72] = blk.reshape(81, 72)
    return xu, wsa, xws, wtsd, wbh, wbnh


def kernel(x, weight):
    x = np.asarray(x, dtype=np.float32)
    weight = np.asarray(weight, dtype=np.float32)
    from concourse.bass_utils import run_bass_kernel_spmd

    xu, wsa, xws, wtsd, wbh, wbnh = _host_prep(x, weight)

    if "nc" not in _CACHE:
        _CACHE["nc"] = _build_program()
    nc = _CACHE["nc"]

    in_maps = []
    for r in range(N_CORES):
        sl = slice(r * S, (r + 1) * S)
        in_maps.append({
            "XU": np.ascontiguousarray(xu[sl]),
            "WSA": np.ascontiguousarray(wsa[sl]),
            "XWS": np.ascontiguousarray(xws[sl]),
            "WTSD": wtsd,
            "WBH": np.ascontiguousarray(wbh[sl]),
            "WBNH": np.ascontiguousarray(wbnh[sl]),
        })
    res = run_bass_kernel_spmd(
        nc, in_maps, list(range(N_CORES)), trace=_CACHE.get("trace", False)
    )
    _CACHE["last_result"] = res
    ys = np.concatenate(
        [np.asarray(res.results[r]["YS"]) for r in range(N_CORES)], axis=0
    )
    # YS (t,u,i,c) -> out_w [V,4,3,3,2] (u,i,t,c)
    out_w = ys.reshape(V, 3, 4, 3, 2).transpose(0, 2, 3, 1, 4)
    out = np.concatenate([x[0, :, :4], out_w], axis=1)[None]
    return out.astype(np.float32)



# revision 6
# speedup vs baseline: 1.3738x; 1.3738x over previous
"""Trainium2 Bass kernel for nn_LConvBilin (lattice gauge bilinear conv).

fp16 redesign, 8-core SPMD: V=16384 sites split contiguously across 8 cores
(2048 sites/core, 16 tiles of 128 sites on SBUF partitions).

DVE tensor_tensor runs at 2 elem/cycle in fp16 when every operand's innermost
AP dim is stride-1, even-length, and 4B-aligned; all product layouts here are
engineered for that. Reductions run at 1x regardless, so products are merged
into few large TTs and reduces kept minimal.

Per-tile pipeline:
  P1 (DVE): one TT [c',a,i | m,k | t,j] over supertiles U2D x WT2 (term-pairs
      t = {uR-part, uI-part} with signs folded host-side), one X-reduce over
      (t,j) -> V [c',a,i,m,k].
  V2 (ACT): 4 strided copies arranging V into [c'o,a,m,i,tk] term-pair form.
  P2 (DVE): UP2R = DMA-broadcast of UP2 over (m,i); one TT V2 x UP2R; two
      X-reduces (per c'o) writing the transport block of TALL directly.
  stage C (ACT+PE): gather TALL into [128,121] pair+diag blocks, PE-transpose
      (fp16), 6 matmuls against host-folded weight matrix -> M in PSUM; ACT
      evacuates to MS [c,t,u,v(pad 10),j] fp16.
  stage E (DVE): 8 TTs (bcast w2-variants x MS slices) into EBIG, one XY-reduce
      over (term, v, j) for both c' at once, one TT adding the unit term.

kernel(x, weight) takes FULL inputs, returns FULL output.
"""
import re
import sys

import numpy as np

sys.path.insert(0, "/opt/trn_rl_repo")

DIMS = (16, 16, 8, 8)
V = 16384
N_CORES = 8
S = V // N_CORES
NT = S // 128
PAIRS = [(0, 1), (0, 2), (1, 2)]

_CACHE = {}
SPLIT_WAITS = True


# ---------------------------------------------------------------- tile fixes
def _apply_tile_fixes():
    """This walrus build allows very few semaphore waits per instruction.
    Split the global-clock drain wait across single-wait sync NOPs."""
    if _CACHE.get("fixed"):
        return
    from concourse.tile import TileContext
    from concourse.vector_clock import ScopedClock, VectorClock

    def _clock_values(vc):
        m = re.match(r"VectorClock\(\[(.*)\]\)", repr(vc))
        return [int(x) for x in m.group(1).split(",")]

    def _drain_and_barrier_split(self, tick_clock, wait_clock):
        vals = _clock_values(tick_clock.global_clock)
        for p, val in [(p, v) for p, v in enumerate(vals) if v > 0]:
            v = VectorClock()
            v.require_at_least(p, val)
            nop_inst = self.nc.sync.nop(nofuse=True, hint="drain_split_wait")
            wait_clock.add_sem_waits(nop_inst.ins, ScopedClock({None: v}))
        self.nc.sync.drain()
        self.nc.all_engine_barrier()
        assert self.sems is not None
        popped = self.nc._tile_sem_poison_stack.pop()
        assert popped is self._sem_poison
        self.nc.clear_and_free_semaphores(list(self.sems.allocated().values()))
        self.nc.all_engine_barrier()

    TileContext._drain_and_barrier = _drain_and_barrier_split
    _CACHE["fixed"] = True


def _split_sync_waits(nc, cap=1):
    import concourse.mybir as mybir

    for fn in nc.m.functions:
        for bb in fn.blocks:
            out = []
            for inst in bb.instructions:
                si = inst.sync_info
                if si is not None and si.on_wait and len(si.on_wait) > cap:
                    waits = list(si.on_wait)
                    for i in range(cap, len(waits), cap):
                        nop = mybir.InstNoOp(
                            name=f"{inst.name}-wsplit{i}", ins=[], outs=[]
                        )
                        nop.engine = inst.engine
                        nop.sync_info = mybir.SyncInfo(
                            on_wait=waits[i : i + cap], on_update=[]
                        )
                        nop.bass_nofuse = True
                        out.append(nop)
                    si.on_wait = waits[:cap]
                out.append(inst)
            bb.instructions = out


# ---------------------------------------------------------------- program
def _build_program():
    import concourse.bass as bass
    import concourse.mybir as mybir
    from concourse.masks import make_identity
    from concourse.tile import TileContext

    _apply_tile_fixes()
    F32 = mybir.dt.float32
    F16 = mybir.dt.float16
    MULT = mybir.AluOpType.mult
    ADD = mybir.AluOpType.add

    nc = bass.Bass()
    U2D = nc.dram_tensor("U2D", [S, 144], F16, kind="ExternalInput")
    WT2 = nc.dram_tensor("WT2", [S, 1728], F16, kind="ExternalInput")
    UP2 = nc.dram_tensor("UP2", [S, 144], F16, kind="ExternalInput")
    XW = nc.dram_tensor("XW", [S, 72], F16, kind="ExternalInput")
    W2S = nc.dram_tensor("W2S", [S, 288], F16, kind="ExternalInput")
    WTS = nc.dram_tensor("WTS", [81, 648], F16, kind="ExternalInput")
    YS = nc.dram_tensor("YS", [S, 72], F16, kind="ExternalOutput")

    def AP(t, off, dims):
        return bass.AP(t.tensor, t.offset + off, [list(t.ap[0])] + dims)

    with TileContext(nc) as tc:
        with (
            nc.allow_low_precision(reason="fp16 kernel, tol 2e-2"),
            tc.tile_pool(name="const", bufs=1) as cpool,
            tc.tile_pool(name="work", bufs=3) as pool,
            tc.tile_pool(name="big", bufs=2) as bigpool,
            tc.tile_pool(name="ps_tr", bufs=2, space="PSUM") as ps_tr,
            tc.tile_pool(name="ps_mm", bufs=2, space="PSUM") as ps_mm,
        ):
            identf = cpool.tile([128, 128], F32)
            make_identity(nc, identf[:, :])
            idf = cpool.tile([128, 128], F16)
            nc.scalar.copy(idf[:, :], identf[:, :])
            wtsb = cpool.tile([81, 648], F16)
            nc.sync.dma_start(wtsb[:, :], WTS[:, :])

            def emit_tile(t):
                rows = slice(t * 128, (t + 1) * 128)
                u2d = pool.tile([128, 144], F16, tag="u2d")
                wt2 = bigpool.tile([128, 1728], F16, tag="wt2")
                up2 = pool.tile([128, 144], F16, tag="up2")
                w2s = pool.tile([128, 288], F16, tag="w2s")
                tall = pool.tile([128, 360], F16, tag="tall")
                nc.sync.dma_start(u2d[:, :], U2D[rows, :])
                nc.sync.dma_start(wt2[:, :], WT2[rows, :])
                nc.sync.dma_start(up2[:, :], UP2[rows, :])
                nc.sync.dma_start(w2s[:, :], W2S[rows, :])
                nc.sync.dma_start(tall[:, 0:72], XW[rows, :])

                # UP2R [c'o,a,(m,i)rep12,l,tk] via SBUF->SBUF broadcast DMA
                up2r = bigpool.tile([128, 1728], F16, tag="up2r")
                for g in range(8):
                    nc.sync.dma_start(
                        AP(up2r, 216 * g, [[18, 12], [1, 18]]),
                        AP(up2, 18 * g, [[0, 12], [1, 18]]),
                    )

                # ---- P1: one TT + one reduce ----
                prod1 = bigpool.tile([128, 1728], F16, tag="prod1")
                nc.vector.tensor_tensor(
                    out=AP(prod1, 0, [[72, 24], [6, 12], [1, 6]]),
                    in0=AP(u2d, 0, [[6, 24], [0, 12], [1, 6]]),
                    in1=AP(wt2, 0, [[72, 24], [6, 12], [1, 6]]),
                    op=MULT,
                )
                # reduce writes vv in m-major [c'][a][m][i][k] so the V2
                # copies can flatten (a,m) into 3-dim ACT APs
                vv = pool.tile([128, 288], F16, tag="vv")
                nc.vector.tensor_reduce(
                    out=AP(vv, 0, [[36, 8], [3, 3], [9, 4], [1, 3]]),
                    in_=AP(prod1, 0, [[6, 288], [1, 6]]),
                    axis=mybir.AxisListType.X,
                    op=ADD,
                )

                # ---- V2 build [c'o,a,m,i,tk] (ACT, 4 copies) ----
                # vv strides: k1(3) i3(3) m9(4) a36(4) c'144(2)
                # v2 strides: tk1(6) i6(3) m18(4) a72(4) c'o288(2)
                v2 = pool.tile([128, 576], F16, tag="v2")
                for co, tt_, voff in (
                    (0, 0, 0), (0, 1, 144), (1, 0, 144), (1, 1, 0)
                ):
                    nc.scalar.copy(
                        AP(v2, 288 * co + 3 * tt_, [[18, 16], [6, 3], [1, 3]]),
                        AP(vv, voff, [[9, 16], [3, 3], [1, 3]]),
                    )

                # ---- P2: one TT + two reduces into TALL T-part ----
                prod2 = bigpool.tile([128, 1728], F16, tag="prod2")
                nc.vector.tensor_tensor(
                    out=AP(prod2, 0, [[18, 96], [6, 3], [1, 6]]),
                    in0=AP(v2, 0, [[6, 96], [0, 3], [1, 6]]),
                    in1=AP(up2r, 0, [[18, 96], [6, 3], [1, 6]]),
                    op=MULT,
                )
                # TALL [ch20][r3][c2][q3]; T-part off = 72 + 72a+18m+6i+3c'+l
                for co in range(2):
                    nc.vector.tensor_reduce(
                        out=AP(tall, 72 + 3 * co, [[18, 16], [6, 3], [1, 3]]),
                        in_=AP(prod2, 864 * co,
                               [[54, 16], [18, 3], [6, 3], [1, 6]]),
                        axis=mybir.AxisListType.X,
                        op=ADD,
                    )

                # ---- gathers -> [128,121] blocks, PE transpose, evac ----
                tqp, tqd = [], []
                for pi, (p_, P_) in enumerate(PAIRS):
                    gq = pool.tile([128, 121], F16, tag=f"gq{pi}")
                    nc.scalar.copy(
                        AP(gq, 0, [[1, 80]]),
                        AP(tall, 6 * p_ + P_,
                           [[3, 2], [5 * (P_ - p_), 2], [18, 20]]),
                    )
                    nc.scalar.copy(
                        AP(gq, 80, [[1, 40]]),
                        AP(tall, 7 * pi, [[3, 2], [18, 20]]),
                    )
                    nc.gpsimd.memset(gq[:, 120:121], 1.0)
                    ptp = ps_tr.tile([128, 128], F16, tag="ptp")
                    nc.tensor.transpose(ptp[0:80, :], gq[:, 0:80], idf[:, :])
                    sp = pool.tile([80, 128], F16, tag=f"tqp{pi}")
                    nc.scalar.copy(sp[:, :], ptp[0:80, :])
                    ptd = ps_tr.tile([64, 128], F16, tag="ptd")
                    nc.tensor.transpose(ptd[0:41, :], gq[:, 80:121], idf[:, :])
                    sd = pool.tile([41, 128], F16, tag=f"tqd{pi}")
                    nc.scalar.copy(sd[:, :], ptd[0:41, :])
                    tqp.append(sp)
                    tqd.append(sd)

                # ---- stage C: 6 matmuls; MS [c2][t3][u4][v10][j3] = 720 ----
                ms = bigpool.tile([128, 720], F16, tag="ms")
                for pi, (p_, P_) in enumerate(PAIRS):
                    mm = ps_mm.tile([128, 144], F32, tag="mmp")
                    nc.tensor.matmul(
                        mm[:, :], tqp[pi][0:80, :],
                        wtsb[0:80, 144 * pi : 144 * pi + 144],
                        start=True, stop=True,
                    )
                    for ordv, (r, q) in enumerate(((p_, P_), (P_, p_))):
                        nc.scalar.copy(
                            AP(ms, 120 * q + r, [[30, 4], [3, 9], [360, 2]]),
                            AP(mm, 72 * ordv, [[1, 72]]),
                        )
                for r in range(3):
                    mm = ps_mm.tile([128, 72], F32, tag="mmd")
                    nc.tensor.matmul(
                        mm[:, :], tqd[r][0:41, :],
                        wtsb[0:41, 432 + 72 * r : 504 + 72 * r],
                        start=True, stop=True,
                    )
                    nc.scalar.copy(
                        AP(ms, 120 * r + r, [[30, 4], [3, 9], [360, 2]]),
                        AP(mm, 0, [[1, 72]]),
                    )

                # ---- stage E: 8 TTs + 1 XY-reduce + unit add ----
                ebig = bigpool.tile([128, 3456], F16, tag="ebig")
                cms = [0, 1, 0, 1, 1, 0, 1, 0]
                vbs = [0, 0, 4, 4, 0, 0, 4, 4]
                for k_ in range(8):
                    nc.vector.tensor_tensor(
                        out=AP(ebig, 432 * k_, [[12, 36], [1, 12]]),
                        in0=AP(w2s, 36 * k_, [[0, 12], [12, 3], [1, 12]]),
                        in1=AP(ms, 360 * cms[k_] + 3 * vbs[k_],
                               [[30, 12], [0, 3], [1, 12]]),
                        op=MULT,
                    )
                out2 = pool.tile([128, 72], F16, tag="out2")
                nc.vector.tensor_reduce(
                    out=AP(out2, 0, [[36, 2], [1, 36]]),
                    in_=AP(ebig, 0, [[1728, 2], [12, 36], [432, 4], [1, 12]]),
                    axis=mybir.AxisListType.XY,
                    op=ADD,
                )
                nc.vector.tensor_tensor(
                    out=AP(out2, 0, [[36, 2], [12, 3], [3, 4], [1, 3]]),
                    in0=AP(out2, 0, [[36, 2], [12, 3], [3, 4], [1, 3]]),
                    in1=AP(ms, 24, [[360, 2], [120, 3], [30, 4], [1, 3]]),
                    op=ADD,
                )
                nc.sync.dma_start(YS[rows, :], out2[:, :])

            for t in range(NT):
                emit_tile(t)
    if SPLIT_WAITS:
        _split_sync_waits(nc)
    return nc


# ---------------------------------------------------------------- host prep
def _host_prep(x, weight):
    x = np.ascontiguousarray(x, dtype=np.float32)
    weight = np.ascontiguousarray(weight, dtype=np.float32)
    u = x[0, :, :4]          # [V, a, i, j, c]
    w = x[0, :, 4:]          # [V, m, i, j, c]
    wgrid = w.reshape(DIMS + (4, 3, 3, 2))
    ws = np.stack([np.roll(wgrid, -1, axis=a).reshape(V, 4, 3, 3, 2)
                   for a in range(4)], axis=1)       # [V, a, m, j, k, c]

    uu = np.stack([u[..., 0], u[..., 1]], axis=-2)   # [V,a,i,t,j]
    u2d = np.broadcast_to(uu[:, None], (V, 2, 4, 3, 2, 3)).reshape(V, 144)

    wsR, wsI = ws[..., 0], ws[..., 1]                # [V,a,m,j,k]
    re_blk = np.stack([wsR, -wsI], axis=-3)          # [V,a,m,t,j,k]
    im_blk = np.stack([wsI, wsR], axis=-3)
    wt2 = np.stack([re_blk, im_blk], axis=1)         # [V,c',a,m,t,j,k]
    wt2 = wt2.transpose(0, 1, 2, 3, 6, 4, 5)         # [V,c',a,m,k,t,j]
    wt2 = np.broadcast_to(
        wt2[:, :, :, None], (V, 2, 4, 3, 4, 3, 2, 3)
    ).reshape(V, 1728)

    uR, uI = u[..., 0], u[..., 1]                    # [V,a,l,k]
    up2 = np.stack([
        np.stack([uR, uI], axis=-2),
        np.stack([uR, -uI], axis=-2),
    ], axis=1).reshape(V, 144)

    xw = w.transpose(0, 1, 2, 4, 3).reshape(V, 72)   # [V,m,i,c,k]

    wR, wI = w[..., 0], w[..., 1]                    # [V, m, i, j]
    wb = wR.transpose(0, 2, 1, 3)                    # [V,i,v,j] = wR[v,i,j]
    wbI = wI.transpose(0, 2, 1, 3)
    wbd = wR.transpose(0, 3, 1, 2)
    wbdI = wI.transpose(0, 3, 1, 2)
    w2s = np.stack(
        [wb, -wbI, wbd, wbdI, wb, wbI, wbd, -wbdI], axis=1
    ).reshape(V, 288)

    # WTS: baseline wtsd with columns reordered into pair/diag groups
    o1 = weight[:, :, :20]; o2 = weight[:, :, 20:40]; o3 = weight[:, :, 40]
    wtsd = np.zeros((81, 648), np.float32)
    for r in range(3):
        for q in range(3):
            rq = r * 3 + q
            blk = np.zeros((81, 4, 9, 2), np.float32)
            if r == q:
                for c in range(2):
                    sgn = 1.0 if c == 0 else -1.0
                    for ch in range(20):
                        blk[c * 20 + ch, :, :, c] = (
                            o1[:, :, ch] + sgn * o2[:, :, ch])
                blk[40, :, :, 0] = o3
            else:
                p_, P_ = min(r, q), max(r, q)
                my_ord = 0 if (r, q) == (p_, P_) else 1
                for c in range(2):
                    sgn = 1.0 if c == 0 else -1.0
                    for ch in range(20):
                        blk[c * 40 + my_ord * 20 + ch, :, :, c] = o1[:, :, ch]
                        blk[c * 40 + (1 - my_ord) * 20 + ch, :, :, c] = (
                            sgn * o2[:, :, ch])
            wtsd[:, rq * 72:(rq + 1) * 72] = blk.reshape(81, 72)
    order = [1, 3, 2, 6, 5, 7, 0, 4, 8]   # (01,10),(02,20),(12,21),00,11,22
    wts = np.concatenate(
        [wtsd[:, rq * 72:(rq + 1) * 72] for rq in order], axis=1)

    f16 = np.float16
    return (u2d.astype(f16), wt2.astype(f16), up2.astype(f16),
            xw.astype(f16), w2s.astype(f16), wts.astype(f16))


def kernel(x, weight):
    x = np.asarray(x, dtype=np.float32)
    weight = np.asarray(weight, dtype=np.float32)
    from concourse.bass_utils import run_bass_kernel_spmd

    u2d, wt2, up2, xw, w2s, wts = _host_prep(x, weight)

    if "nc" not in _CACHE:
        _CACHE["nc"] = _build_program()
    nc = _CACHE["nc"]

    in_maps = []
    for r in range(N_CORES):
        sl = slice(r * S, (r + 1) * S)
        in_maps.append({
            "U2D": np.ascontiguousarray(u2d[sl]),
            "WT2": np.ascontiguousarray(wt2[sl]),
            "UP2": np.ascontiguousarray(up2[sl]),
            "XW": np.ascontiguousarray(xw[sl]),
            "W2S": np.ascontiguousarray(w2s[sl]),
            "WTS": wts,
        })
    res = run_bass_kernel_spmd(
        nc, in_maps, list(range(N_CORES)), trace=_CACHE.get("trace", False)
    )
    _CACHE["last_result"] = res
    ys = np.concatenate(
        [np.asarray(res.results[r]["YS"]) for r in range(N_CORES)], axis=0
    ).astype(np.float32)
    # ys [V, c'2, t3, u4, i3] -> out_w [V, u, i, k=t, c']
    out_w = ys.reshape(V, 2, 3, 4, 3).transpose(0, 3, 4, 2, 1)
    out = np.concatenate([x[0, :, :4], out_w], axis=1)[None]
    return out.astype(np.float32)


# revision 10
# speedup vs baseline: 1.5568x; 1.1332x over previous
"""Trainium2 Bass kernel for nn_LConvBilin (lattice gauge bilinear conv).

fp16 redesign, 8-core SPMD: V=16384 sites split contiguously across 8 cores
(2048 sites/core, 16 tiles of 128 sites on SBUF partitions).

DVE tensor_tensor runs at 2 elem/cycle in fp16 when every operand's innermost
AP dim is stride-1, even-length, and 4B-aligned; all product layouts here are
engineered for that. Reductions run at 1x regardless, so products are merged
into few large TTs and reduces kept minimal.

Per-tile pipeline:
  P1 (DVE): one TT [c',a,i | m,k | t,j] over supertiles U2D x WT2 (term-pairs
      t = {uR-part, uI-part} with signs folded host-side), one X-reduce over
      (t,j) -> V [c',a,i,m,k].
  V2 (ACT): 4 strided copies arranging V into [c'o,a,m,i,tk] term-pair form.
  P2 (DVE): UP2R = DMA-broadcast of UP2 over (m,i); one TT V2 x UP2R; two
      X-reduces (per c'o) writing the transport block of TALL directly.
  stage C (ACT+PE): gather TALL into [128,121] pair+diag blocks, PE-transpose
      (fp16), 6 matmuls against host-folded weight matrix -> M in PSUM; ACT
      evacuates to MS [c,t,u,v(pad 10),j] fp16.
  stage E (DVE): 8 TTs (bcast w2-variants x MS slices) into EBIG, one XY-reduce
      over (term, v, j) for both c' at once, one TT adding the unit term.

kernel(x, weight) takes FULL inputs, returns FULL output.
"""
import re
import sys

import numpy as np

sys.path.insert(0, "/opt/trn_rl_repo")

DIMS = (16, 16, 8, 8)
V = 16384
N_CORES = 8
S = V // N_CORES
NT = S // 128
PAIRS = [(0, 1), (0, 2), (1, 2)]

_CACHE = {}
SPLIT_WAITS = True


# ---------------------------------------------------------------- tile fixes
def _apply_tile_fixes():
    """This walrus build allows very few semaphore waits per instruction.
    Split the global-clock drain wait across single-wait sync NOPs."""
    if _CACHE.get("fixed"):
        return
    from concourse.tile import TileContext
    from concourse.vector_clock import ScopedClock, VectorClock

    def _clock_values(vc):
        m = re.match(r"VectorClock\(\[(.*)\]\)", repr(vc))
        return [int(x) for x in m.group(1).split(",")]

    def _drain_and_barrier_split(self, tick_clock, wait_clock):
        vals = _clock_values(tick_clock.global_clock)
        for p, val in [(p, v) for p, v in enumerate(vals) if v > 0]:
            v = VectorClock()
            v.require_at_least(p, val)
            nop_inst = self.nc.sync.nop(nofuse=True, hint="drain_split_wait")
            wait_clock.add_sem_waits(nop_inst.ins, ScopedClock({None: v}))
        self.nc.sync.drain()
        self.nc.all_engine_barrier()
        assert self.sems is not None
        popped = self.nc._tile_sem_poison_stack.pop()
        assert popped is self._sem_poison
        self.nc.clear_and_free_semaphores(list(self.sems.allocated().values()))
        self.nc.all_engine_barrier()

    TileContext._drain_and_barrier = _drain_and_barrier_split
    _CACHE["fixed"] = True


def _split_sync_waits(nc, cap=1):
    import concourse.mybir as mybir

    for fn in nc.m.functions:
        for bb in fn.blocks:
            out = []
            for inst in bb.instructions:
                si = inst.sync_info
                if si is not None and si.on_wait and len(si.on_wait) > cap:
                    waits = list(si.on_wait)
                    for i in range(cap, len(waits), cap):
                        nop = mybir.InstNoOp(
                            name=f"{inst.name}-wsplit{i}", ins=[], outs=[]
                        )
                        nop.engine = inst.engine
                        nop.sync_info = mybir.SyncInfo(
                            on_wait=waits[i : i + cap], on_update=[]
                        )
                        nop.bass_nofuse = True
                        out.append(nop)
                    si.on_wait = waits[:cap]
                out.append(inst)
            bb.instructions = out


# ---------------------------------------------------------------- program
def _build_program():
    import concourse.bass as bass
    import concourse.mybir as mybir
    from concourse.masks import make_identity
    from concourse.tile import TileContext

    _apply_tile_fixes()
    F32 = mybir.dt.float32
    F16 = mybir.dt.float16
    MULT = mybir.AluOpType.mult
    ADD = mybir.AluOpType.add

    nc = bass.Bass()
    U2D = nc.dram_tensor("U2D", [S, 144], F16, kind="ExternalInput")
    WT2 = nc.dram_tensor("WT2", [S, 1728], F16, kind="ExternalInput")
    UP2 = nc.dram_tensor("UP2", [S, 144], F16, kind="ExternalInput")
    XW = nc.dram_tensor("XW", [S, 72], F16, kind="ExternalInput")
    W2S = nc.dram_tensor("W2S", [S, 288], F16, kind="ExternalInput")
    WTS = nc.dram_tensor("WTS", [81, 648], F16, kind="ExternalInput")
    YS = nc.dram_tensor("YS", [S, 72], F16, kind="ExternalOutput")

    def AP(t, off, dims):
        return bass.AP(t.tensor, t.offset + off, [list(t.ap[0])] + dims)

    with TileContext(nc) as tc:
        with (
            nc.allow_low_precision(reason="fp16 kernel, tol 2e-2"),
            tc.tile_pool(name="const", bufs=1) as cpool,
            tc.tile_pool(name="work", bufs=3) as pool,
            tc.tile_pool(name="big", bufs=2) as bigpool,
            tc.tile_pool(name="ps_tr", bufs=2, space="PSUM") as ps_tr,
            tc.tile_pool(name="ps_mm", bufs=2, space="PSUM") as ps_mm,
        ):
            identf = cpool.tile([128, 128], F32)
            make_identity(nc, identf[:, :])
            idf = cpool.tile([128, 128], F16)
            nc.scalar.copy(idf[:, :], identf[:, :])
            wtsb = cpool.tile([81, 648], F16)
            nc.sync.dma_start(wtsb[:, :], WTS[:, :])

            def emit_tile(t):
                rows = slice(t * 128, (t + 1) * 128)
                u2d = pool.tile([128, 144], F16, tag="u2d")
                wt2 = bigpool.tile([128, 1728], F16, tag="wt2")
                up2 = pool.tile([128, 144], F16, tag="up2")
                w2s = pool.tile([128, 288], F16, tag="w2s")
                tall = pool.tile([128, 360], F16, tag="tall")
                nc.sync.dma_start(u2d[:, :], U2D[rows, :])
                nc.sync.dma_start(wt2[:, :], WT2[rows, :])
                nc.sync.dma_start(up2[:, :], UP2[rows, :])
                nc.sync.dma_start(w2s[:, :], W2S[rows, :])
                nc.sync.dma_start(tall[:, 0:72], XW[rows, :])

                # UP2R [c'o,a,(m,i)rep12,l,tk] via SBUF->SBUF broadcast DMA


                # ---- P1: one TT + one reduce ----
                prod1 = bigpool.tile([128, 1728], F16, tag="prod1")
                nc.vector.tensor_tensor(
                    out=AP(prod1, 0, [[72, 24], [6, 12], [1, 6]]),
                    in0=AP(u2d, 0, [[6, 24], [0, 12], [1, 6]]),
                    in1=AP(wt2, 0, [[72, 24], [6, 12], [1, 6]]),
                    op=MULT,
                )
                # reduce writes vv in m-major [c'][a][m][i][k] so the V2
                # copies can flatten (a,m) into 3-dim ACT APs
                vv = pool.tile([128, 288], F16, tag="vv")
                nc.vector.tensor_reduce(
                    out=AP(vv, 0, [[36, 8], [3, 3], [9, 4], [1, 3]]),
                    in_=AP(prod1, 0, [[6, 288], [1, 6]]),
                    axis=mybir.AxisListType.X,
                    op=ADD,
                )

                # ---- V2 build [c'o,a,m,i,tk] (ACT, 4 copies) ----
                # vv strides: k1(3) i3(3) m9(4) a36(4) c'144(2)
                # v2 strides: tk1(6) i6(3) m18(4) a72(4) c'o288(2)
                v2 = pool.tile([128, 576], F16, tag="v2")
                for co, tt_, voff in (
                    (0, 0, 0), (0, 1, 144), (1, 0, 144), (1, 1, 0)
                ):
                    nc.scalar.copy(
                        AP(v2, 288 * co + 3 * tt_, [[18, 16], [6, 3], [1, 3]]),
                        AP(vv, voff, [[9, 16], [3, 3], [1, 3]]),
                    )

                # ---- P2: one TT (up2 broadcast over (m,i) via stride-0)
                #      + two reduces into TALL T-part ----
                prod2 = bigpool.tile([128, 1728], F16, tag="prod2")
                nc.vector.tensor_tensor(
                    out=AP(prod2, 0, [[216, 8], [18, 12], [6, 3], [1, 6]]),
                    in0=AP(v2, 0, [[72, 8], [6, 12], [0, 3], [1, 6]]),
                    in1=AP(up2, 0, [[18, 8], [0, 12], [6, 3], [1, 6]]),
                    op=MULT,
                )
                # TALL [ch20][r3][c2][q3]; T-part off = 72 + 72a+18m+6i+3c'+l
                for co in range(2):
                    nc.vector.tensor_reduce(
                        out=AP(tall, 72 + 3 * co, [[18, 16], [6, 3], [1, 3]]),
                        in_=AP(prod2, 864 * co,
                               [[54, 16], [18, 3], [6, 3], [1, 6]]),
                        axis=mybir.AxisListType.X,
                        op=ADD,
                    )

                # ---- gathers -> [128,121] blocks, PE transpose, evac ----
                tqp, tqd = [], []
                for pi, (p_, P_) in enumerate(PAIRS):
                    gq = pool.tile([128, 121], F16, tag=f"gq{pi}")
                    nc.scalar.copy(
                        AP(gq, 0, [[1, 80]]),
                        AP(tall, 6 * p_ + P_,
                           [[3, 2], [5 * (P_ - p_), 2], [18, 20]]),
                    )
                    nc.scalar.copy(
                        AP(gq, 80, [[1, 40]]),
                        AP(tall, 7 * pi, [[3, 2], [18, 20]]),
                    )
                    nc.gpsimd.memset(gq[:, 120:121], 1.0)
                    ptp = ps_tr.tile([128, 128], F16, tag="ptp")
                    nc.tensor.transpose(ptp[0:80, :], gq[:, 0:80], idf[:, :])
                    sp = pool.tile([80, 128], F16, tag=f"tqp{pi}")
                    nc.scalar.copy(sp[:, :], ptp[0:80, :])
                    ptd = ps_tr.tile([64, 128], F16, tag="ptd")
                    nc.tensor.transpose(ptd[0:41, :], gq[:, 80:121], idf[:, :])
                    sd = pool.tile([41, 128], F16, tag=f"tqd{pi}")
                    nc.scalar.copy(sd[:, :], ptd[0:41, :])
                    tqp.append(sp)
                    tqd.append(sd)

                # ---- stage C: 6 matmuls; MS [c2][t3][u4][v10][j3] = 720 ----
                ms = bigpool.tile([128, 720], F16, tag="ms")
                for pi, (p_, P_) in enumerate(PAIRS):
                    mm = ps_mm.tile([128, 144], F32, tag="mmp")
                    nc.tensor.matmul(
                        mm[:, :], tqp[pi][0:80, :],
                        wtsb[0:80, 144 * pi : 144 * pi + 144],
                        start=True, stop=True,
                    )
                    for ordv, (r, q) in enumerate(((p_, P_), (P_, p_))):
                        nc.scalar.copy(
                            AP(ms, 120 * q + r, [[30, 4], [3, 9], [360, 2]]),
                            AP(mm, 72 * ordv, [[1, 72]]),
                        )
                for r in range(3):
                    mm = ps_mm.tile([128, 72], F32, tag="mmd")
                    nc.tensor.matmul(
                        mm[:, :], tqd[r][0:41, :],
                        wtsb[0:41, 432 + 72 * r : 504 + 72 * r],
                        start=True, stop=True,
                    )
                    nc.scalar.copy(
                        AP(ms, 120 * r + r, [[30, 4], [3, 9], [360, 2]]),
                        AP(mm, 0, [[1, 72]]),
                    )

                return rows, w2s, ms

            def emit_back(state):
                # ---- stage E: rebig half on DVE, imbig half on GpSimd ----
                rows, w2s, ms = state
                ebig = bigpool.tile([128, 3456], F16, tag="ebig")
                cms = [0, 1, 0, 1, 1, 0, 1, 0]
                vbs = [0, 0, 4, 4, 0, 0, 4, 4]
                for k_ in range(8):
                    eng = nc.vector if k_ < 4 else nc.gpsimd
                    eng.tensor_tensor(
                        out=AP(ebig, 432 * k_, [[12, 36], [1, 12]]),
                        in0=AP(w2s, 36 * k_, [[0, 12], [12, 3], [1, 12]]),
                        in1=AP(ms, 360 * cms[k_] + 3 * vbs[k_],
                               [[30, 12], [0, 3], [1, 12]]),
                        op=MULT,
                    )
                # imbig: one gpsimd tree-fold, DVE finishes both reductions
                efold = pool.tile([128, 864], F16, tag="efold")
                nc.gpsimd.tensor_tensor(
                    out=AP(efold, 0, [[1, 864]]),
                    in0=AP(ebig, 1728, [[1, 864]]),
                    in1=AP(ebig, 2592, [[1, 864]]),
                    op=ADD,
                )
                out2 = pool.tile([128, 72], F16, tag="out2")
                nc.vector.tensor_reduce(
                    out=AP(out2, 0, [[1, 36]]),
                    in_=AP(ebig, 0, [[12, 36], [432, 4], [1, 12]]),
                    axis=mybir.AxisListType.XY,
                    op=ADD,
                )
                nc.vector.tensor_reduce(
                    out=AP(out2, 36, [[1, 36]]),
                    in_=AP(efold, 0, [[12, 36], [432, 2], [1, 12]]),
                    axis=mybir.AxisListType.XY,
                    op=ADD,
                )
                nc.vector.tensor_tensor(
                    out=AP(out2, 0, [[36, 2], [12, 3], [3, 4], [1, 3]]),
                    in0=AP(out2, 0, [[36, 2], [12, 3], [3, 4], [1, 3]]),
                    in1=AP(ms, 24, [[360, 2], [120, 3], [30, 4], [1, 3]]),
                    op=ADD,
                )
                nc.sync.dma_start(YS[rows, :], out2[:, :])

            prev = None
            for t in range(NT):
                st = emit_tile(t)
                if prev is not None:
                    emit_back(prev)
                prev = st
            emit_back(prev)
    if SPLIT_WAITS:
        _split_sync_waits(nc)
    return nc


# ---------------------------------------------------------------- host prep
def _host_prep(x, weight):
    x = np.ascontiguousarray(x, dtype=np.float32)
    weight = np.ascontiguousarray(weight, dtype=np.float32)
    u = x[0, :, :4]          # [V, a, i, j, c]
    w = x[0, :, 4:]          # [V, m, i, j, c]
    wgrid = w.reshape(DIMS + (4, 3, 3, 2))
    ws = np.stack([np.roll(wgrid, -1, axis=a).reshape(V, 4, 3, 3, 2)
                   for a in range(4)], axis=1)       # [V, a, m, j, k, c]

    uu = np.stack([u[..., 0], u[..., 1]], axis=-2)   # [V,a,i,t,j]
    u2d = np.broadcast_to(uu[:, None], (V, 2, 4, 3, 2, 3)).reshape(V, 144)

    wsR, wsI = ws[..., 0], ws[..., 1]                # [V,a,m,j,k]
    re_blk = np.stack([wsR, -wsI], axis=-3)          # [V,a,m,t,j,k]
    im_blk = np.stack([wsI, wsR], axis=-3)
    wt2 = np.stack([re_blk, im_blk], axis=1)         # [V,c',a,m,t,j,k]
    wt2 = wt2.transpose(0, 1, 2, 3, 6, 4, 5)         # [V,c',a,m,k,t,j]
    wt2 = np.broadcast_to(
        wt2[:, :, :, None], (V, 2, 4, 3, 4, 3, 2, 3)
    ).reshape(V, 1728)

    uR, uI = u[..., 0], u[..., 1]                    # [V,a,l,k]
    up2 = np.stack([
        np.stack([uR, uI], axis=-2),
        np.stack([uR, -uI], axis=-2),
    ], axis=1).reshape(V, 144)

    xw = w.transpose(0, 1, 2, 4, 3).reshape(V, 72)   # [V,m,i,c,k]

    wR, wI = w[..., 0], w[..., 1]                    # [V, m, i, j]
    wb = wR.transpose(0, 2, 1, 3)                    # [V,i,v,j] = wR[v,i,j]
    wbI = wI.transpose(0, 2, 1, 3)
    wbd = wR.transpose(0, 3, 1, 2)
    wbdI = wI.transpose(0, 3, 1, 2)
    w2s = np.stack(
        [wb, -wbI, wbd, wbdI, wb, wbI, wbd, -wbdI], axis=1
    ).reshape(V, 288)

    # WTS: baseline wtsd with columns reordered into pair/diag groups
    o1 = weight[:, :, :20]; o2 = weight[:, :, 20:40]; o3 = weight[:, :, 40]
    wtsd = np.zeros((81, 648), np.float32)
    for r in range(3):
        for q in range(3):
            rq = r * 3 + q
            blk = np.zeros((81, 4, 9, 2), np.float32)
            if r == q:
                for c in range(2):
                    sgn = 1.0 if c == 0 else -1.0
                    for ch in range(20):
                        blk[c * 20 + ch, :, :, c] = (
                            o1[:, :, ch] + sgn * o2[:, :, ch])
                blk[40, :, :, 0] = o3
            else:
                p_, P_ = min(r, q), max(r, q)
                my_ord = 0 if (r, q) == (p_, P_) else 1
                for c in range(2):
                    sgn = 1.0 if c == 0 else -1.0
                    for ch in range(20):
                        blk[c * 40 + my_ord * 20 + ch, :, :, c] = o1[:, :, ch]
                        blk[c * 40 + (1 - my_ord) * 20 + ch, :, :, c] = (
                            sgn * o2[:, :, ch])
            wtsd[:, rq * 72:(rq + 1) * 72] = blk.reshape(81, 72)
    order = [1, 3, 2, 6, 5, 7, 0, 4, 8]   # (01,10),(02,20),(12,21),00,11,22
    wts = np.concatenate(
        [wtsd[:, rq * 72:(rq + 1) * 72] for rq in order], axis=1)

    f16 = np.float16
    return (u2d.astype(f16), wt2.astype(f16), up2.astype(f16),
            xw.astype(f16), w2s.astype(f16), wts.astype(f16))


def kernel(x, weight):
    x = np.asarray(x, dtype=np.float32)
    weight = np.asarray(weight, dtype=np.float32)
    from concourse.bass_utils import run_bass_kernel_spmd

    u2d, wt2, up2, xw, w2s, wts = _host_prep(x, weight)

    if "nc" not in _CACHE:
        _CACHE["nc"] = _build_program()
    nc = _CACHE["nc"]

    in_maps = []
    for r in range(N_CORES):
        sl = slice(r * S, (r + 1) * S)
        in_maps.append({
            "U2D": np.ascontiguousarray(u2d[sl]),
            "WT2": np.ascontiguousarray(wt2[sl]),
            "UP2": np.ascontiguousarray(up2[sl]),
            "XW": np.ascontiguousarray(xw[sl]),
            "W2S": np.ascontiguousarray(w2s[sl]),
            "WTS": wts,
        })
    res = run_bass_kernel_spmd(
        nc, in_maps, list(range(N_CORES)), trace=_CACHE.get("trace", False)
    )
    _CACHE["last_result"] = res
    ys = np.concatenate(
        [np.asarray(res.results[r]["YS"]) for r in range(N_CORES)], axis=0
    ).astype(np.float32)
    # ys [V, c'2, t3, u4, i3] -> out_w [V, u, i, k=t, c']
    out_w = ys.reshape(V, 2, 3, 4, 3).transpose(0, 3, 4, 2, 1)
    out = np.concatenate([x[0, :, :4], out_w], axis=1)[None]
    return out.astype(np.float32)
